# revision 82
# baseline (speedup 1.0000x reference)
"""AttnBlock (GroupNorm + single-head self-attention + residual) on 8 TRN2 cores.

Sharding: data-parallel over batch B=8 -> one [64,64,128] image per core.

Per-core kernel design (fp8/bf16, measured 162-212us depending on the chip's
thermal/power state; was 256us f32r baseline):
  - xT/hT/qT/kT are [C=128 partitions, N=4096 free] (channels on partitions).
  - GN stats run on a bf16 copy of xT (half the DMA to first compute); the
    f32 xT used by the residual is DMA'd behind a semaphore so it never
    contends with the stats-gating stream (~110GB/s per queue).
  - Projections and score matmuls run in bf16: 1 cyc/row like f32r, but the
    LDWEIGHTS is a separate instruction that overlaps the previous matmul,
    unlike f32r whose in-matmul weight load serializes (~230ns/MM).
    (fp8 DoubleRow scores via a c-split [64,2,N] layout measured SLOWER:
    sub-128-partition DR matmuls get no 0.5 cyc/row benefit on hw.)
  - Scores land transposed sT[k, q] = kT_chunk.T @ qT so the probability
    matrix is in [k-partition, q-free] layout for the PV contraction.
  - qT is pre-scaled by A_Q = 8*log2e/sqrt(C) so the score PSUM is directly
    the Schraudolph exponent. Softmax exp splits per half-pair across both
    engines: ACT computes exact exp (scale=ln2/8, bias=-M_SHIFT) into fp8;
    DVE computes Schraudolph fp8 bits = sat_u8(max(psum + B_SCH, 0)) via one
    tensor_scalar with a uint8-bitcast output. M_SHIFT=4 centers exp(s-4) in
    e4m3 range (max observed score ~8.3, overflow at 10.05).
  - Score psums are 5 single-bank tiles; oT 2 banks; Z/pop share 1 bank.
  - PV uses fp8 DoubleRowSwInterleave matmuls (one per k-chunk pair); v is
    written pre-interleaved by reversing wv's columns host-side so each
    chunk's psum copy is a stride-2 byte write (copies alternate ACT/DVE).
  - The softmax denominator Z accumulates via all-ones DoubleRow matmuls in
    sub-bursts of 4 spread over the next block's first pairs; each block's
    epilogue is emitted mid-next-block so its rZ/mult never dam up the
    in-order ACT/DVE queues between exps (that stall was worth ~25us).
  - 1/Z = exp(RZ_SCALE*bitcast_i32(Z) + RZ_BIAS) on ACT: a Schraudolph-log
    feeding the exp table (+-3% on the attention path only). ACT Ln would
    thrash activation-table loads (2.7us each); DVE reciprocal is 3us/tile.
  - The out-projection is transposed: stationary wo, moving oT/Z -> output
    in [C, q] layout, so the residual reads xT directly (no x_all DMA) and
    the epilogue is one scalar_tensor_tensor: out = (pop + bo2) + xT, with
    bo2 = bo + bv@wo folded host-side (kills the v-bias adds too). Output
    DMA writes a transposed [C, N] dram tensor; the host transposes back.
  - Dummy 6-row PE transposes paced behind the GN/projection/ramp phases
    keep the HAM activity window busy (a >3.4us PE-idle gap re-throttles
    the array to 1.2GHz for >=3.4us). Just-in-time k/v/q emission ordering
    removed a 13-16us half-clock window during the attention ramp.
"""

import sys

for _p in ("/opt/trn_rl_repo",):
    if _p not in sys.path:
        sys.path.insert(0, _p)

import numpy as np

import concourse.bass as bass
import concourse.tile as tile
from concourse import bacc, bass_utils, mybir
from concourse.bass_utils import run_bass_kernel_spmd
from concourse.tile import add_dep_helper



B, H, W, C = 8, 64, 64, 128
N = H * W  # 4096 positions per image
GROUPS = 32
GSIZE = C // GROUPS  # 4
EPS = 1e-6
NCORES = 8
P = 128
NT = N // P  # 32 k-chunks
QB = 512  # q-block width
NQB = N // QB  # 8
NPAIR = NT // 2  # 16 k-chunk pairs per q-block
SCALE = C ** -0.5
LOG2E = 1.4426950408889634
M_SHIFT = 4.0  # softmax shift: pexp = exp(s - M_SHIFT)
A_Q = 8.0 * LOG2E * SCALE  # baked into qT so score psum = schraudolph exponent
B_SCH = 8.0 * (7.0 - LOG2E * M_SHIFT) + 0.5  # +0.5 compensates trunc-on-convert
ACT_SCALE = 1.0 / (8.0 * LOG2E)  # un-bake A_Q: exp(psum*ACT_SCALE - M_SHIFT)
LN2 = 0.6931471805599453
# 1/Z ~= exp(-ln2*(bits(Z)*2^-23 - 127.0450466)): schraudolph-log feeding the
# exp table (stays in the exp function set; ACT Ln would thrash table loads
# and DVE reciprocal measures ~3us per 512-elem tile). Max rel err ~3%,
# affecting only the attention path (~13% of output norm).
RZ_SCALE = -LN2 / (1 << 23)
RZ_BIAS = LN2 * (127.0 - 0.0450466)

F32 = mybir.dt.float32
BF16 = mybir.dt.bfloat16
F8 = mybir.dt.float8e4
U8 = mybir.dt.uint8
DR = mybir.MatmulPerfMode.DoubleRow
DRSW = mybir.MatmulPerfMode.DoubleRowSwInterleave


def build_nc():
    nc = bacc.Bacc("TRN2", target_bir_lowering=False, debug=False)

    xt_d = nc.dram_tensor("xt", [C, N], F32, kind="ExternalInput")
    xtb_d = nc.dram_tensor("xtb", [C, N], BF16, kind="ExternalInput")
    wq_d = nc.dram_tensor("wq", [C, C], BF16, kind="ExternalInput")
    wk_d = nc.dram_tensor("wk", [C, C], BF16, kind="ExternalInput")
    wv_d = nc.dram_tensor("wv", [C, C], BF16, kind="ExternalInput")
    wo_d = nc.dram_tensor("wo", [C, C], BF16, kind="ExternalInput")
    # one packed constants tensor: [ident | gmask | gns gnb bqs bk bo2]
    # (seven separate small DMAs cost ~600ns of queue-issue time each)
    consts_d = nc.dram_tensor("consts", [P, 2 * P + 5], F32, kind="ExternalInput")
    sel_d = nc.dram_tensor("sel", [P, P], BF16, kind="ExternalInput")
    out_d = nc.dram_tensor("outT", [C, N], F32, kind="ExternalOutput")

    def col(ap_1d):
        # [C] dram -> [C, 1] partition-column view
        return ap_1d.unsqueeze(1)

    with tile.TileContext(nc) as tc:
        with (
            tc.tile_pool(name="persist", bufs=1) as data,
            tc.tile_pool(name="small", bufs=1) as small,
            tc.tile_pool(name="pexp", bufs=NPAIR + 7) as pexppool,
            tc.tile_pool(name="epi", bufs=3) as epipool,
        ):
            # ---- persistent SBUF tiles ----
            xT = data.tile([P, N], F32)  # exact residual (read late)
            xTb = data.tile([P, N], BF16)  # stats + groupnorm input
            hT = data.tile([P, N], BF16)
            # q/k stay bf16: fp8 DoubleRow scores via a c-split [64,2,N]
            # layout measured SLOWER on hw (64-partition DR matmuls use half
            # the array and get no 0.5-cyc/row benefit: 685ns vs 389ns)
            qTs = data.tile([P, N], BF16)  # q, pre-scaled by A_Q
            kT = data.tile([P, N], BF16)
            v_all = data.tile([P, NT, C], F8)

            wq_s = small.tile([C, C], BF16)
            wk_s = small.tile([C, C], BF16)
            wv_s = small.tile([C, C], BF16)
            wo_s = small.tile([C, C], BF16)
            consts_s = small.tile([P, 2 * P + 5], F32)
            ident_s = consts_s[:, 0:P]
            gmask_s = consts_s[:, P : 2 * P]
            gns_s = consts_s[:, 2 * P : 2 * P + 1]
            gnb_s = consts_s[:, 2 * P + 1 : 2 * P + 2]
            bqs_s = consts_s[:, 2 * P + 2 : 2 * P + 3]
            bk_s = consts_s[:, 2 * P + 3 : 2 * P + 4]
            bo2_s = consts_s[:, 2 * P + 4 : 2 * P + 5]
            ones2 = small.tile([P, 2, C], F8)
            sel_s = small.tile([P, P], BF16)
            eps_s = small.tile([C, 1], F32)
            negm_s = small.tile([C, 1], F32)
            rzb_s = small.tile([C, 1], F32)

            # xTb (bf16, half the bytes) gates the GN stats chain: the first
            # two 256-col chunks are small so bn_stats starts ASAP, the rest
            # stream wide. The exact f32 xT is only read by the residual
            # epilogues tens of microseconds later, so it streams afterwards.
            nc.gpsimd.dma_start(consts_s[:], consts_d[:])
            xtb_cuts = [0, 512, 1408, 2304, 3200, N]
            for ci in range(5):
                cs = slice(xtb_cuts[ci], xtb_cuts[ci + 1])
                eng = nc.sync if ci % 2 == 0 else nc.gpsimd
                eng.dma_start(xTb[:, cs], xtb_d[:, cs])
            nc.gpsimd.dma_start(wq_s[:], wq_d[:])
            nc.gpsimd.dma_start(wk_s[:], wk_d[:])
            nc.gpsimd.dma_start(wv_s[:], wv_d[:])
            nc.gpsimd.dma_start(wo_s[:], wo_d[:])
            nc.gpsimd.dma_start(sel_s[:], sel_d[:])
            nc.gpsimd.memset(ones2[:], 1.0)
            nc.vector.memset(eps_s[:], EPS)
            nc.vector.memset(negm_s[:], -M_SHIFT)
            nc.vector.memset(rzb_s[:], RZ_BIAS)

            # ---- phase 1+2: group norm stats straight off the xT DMA ----
            stats = small.tile([P, 8, nc.vector.BN_STATS_DIM], F32)
            with tc.tile_pool(name="tp", bufs=3, space="PSUM") as tpsum:
                stat_is = []
                for j in range(8):
                    si = nc.vector.bn_stats(
                        out=stats[:, j, :], in_=xTb[:, j * 512 : (j + 1) * 512]
                    )
                    stat_is.append(si)
                    if j % 3 != 0:
                        continue
                    # keep the PE's HAM activity monitor busy through the
                    # DVE-bound stats/GN window so the attention matmuls
                    # start at full clock (idle >3.4us re-throttles); one
                    # dummy transpose every ~2us of stats suffices.
                    pt = tpsum.tile([P, P], F32, tag="tp")
                    nc.tensor.transpose(
                        pt[0:6, :], stats[:, j, :], ident_s
                    )
                # f32 xT streams only after the stats-gating xtb is nearly
                # done: both share ~110GB/s per DMA queue and the epilogues
                # that read xT start tens of microseconds later.
                for ci in range(4):
                    cs = slice(ci * N // 4, (ci + 1) * N // 4)
                    eng = nc.sync if ci % 2 == 0 else nc.gpsimd
                    di = eng.dma_start(xT[:, cs], xt_d[:, cs])
                    add_dep_helper(
                        di.ins, stat_is[5].ins, sync=True, reason="xt after xtb"
                    )
                mv = small.tile([P, nc.vector.BN_AGGR_DIM], F32)
                nc.vector.bn_aggr(out=mv[:], in_=stats[:])
                # per-channel [mean, E[x^2]] -> group-averaged via mask matmul
                st2 = small.tile([P, 2], F32)
                nc.vector.tensor_copy(st2[:, 0:1], mv[:, 0:1])
                msq = small.tile([P, 1], F32)
                nc.vector.tensor_mul(msq[:], mv[:, 0:1], mv[:, 0:1])
                nc.vector.tensor_add(st2[:, 1:2], mv[:, 1:2], msq[:])
                gpsum = tpsum.tile([P, 2], F32, tag="tp")
                nc.tensor.matmul(gpsum[:], gmask_s, st2[:])
                gstat = small.tile([P, 2], F32)
                nc.vector.tensor_copy(gstat[:], gpsum[:])

                # var_g = E_g[x^2] - mean_g^2 ; rstd = 1/sqrt(var_g + eps)
                varg = small.tile([P, 1], F32)
                nc.vector.tensor_mul(varg[:], gstat[:, 0:1], gstat[:, 0:1])
                nc.vector.tensor_tensor(
                    varg[:], gstat[:, 1:2], varg[:], mybir.AluOpType.subtract
                )
                nc.scalar.activation(
                    out=varg[:],
                    in_=varg[:],
                    func=mybir.ActivationFunctionType.Sqrt,
                    bias=eps_s[:],
                    scale=1.0,
                )
                rstd = small.tile([P, 1], F32)
                nc.vector.reciprocal(rstd[:], varg[:])
                # h = x * A + Bc with A = rstd*scale, Bc = bias - mean*A
                A_s = small.tile([P, 1], F32)
                nc.vector.tensor_mul(A_s[:], rstd[:], gns_s)
                mA = small.tile([P, 1], F32)
                nc.vector.tensor_mul(mA[:], gstat[:, 0:1], A_s[:])
                Bc_s = small.tile([P, 1], F32)
                nc.vector.tensor_tensor(
                    Bc_s[:], gnb_s, mA[:], mybir.AluOpType.subtract
                )
                # hT (bf16) in 8 chunks; alternate ACT and DVE.  A dummy PE
                # transpose paced behind each chunk keeps the HAM activity
                # window busy through this PE-idle stretch (else the array
                # re-throttles to half clock right as projections start).
                for j in range(8):
                    sl = slice(j * 512, (j + 1) * 512)
                    if j % 2 == 0:
                        hi = nc.scalar.activation(
                            out=hT[:, sl],
                            in_=xTb[:, sl],
                            func=mybir.ActivationFunctionType.Identity,
                            scale=A_s[:],
                            bias=Bc_s[:],
                        )
                    else:
                        # gpsimd is SBUF-only but this op is SBUF->SBUF
                        eng = nc.gpsimd if j % 4 == 1 else nc.vector
                        hi = eng.tensor_scalar(
                            out=hT[:, sl],
                            in0=xTb[:, sl],
                            scalar1=A_s[:],
                            scalar2=Bc_s[:],
                            op0=mybir.AluOpType.mult,
                            op1=mybir.AluOpType.add,
                        )
                    pt = tpsum.tile([P, P], F32, tag="tp")
                    ti = nc.tensor.transpose(
                        pt[0:6, :], stats[:, j, :], ident_s
                    )
                    add_dep_helper(
                        ti.ins, hi.ins, sync=False, reason="ham pace"
                    )

            # ---- phase 3: projections qTs/kT [C,N] bf16, v [pos,C] fp8 ----
            with (
                tc.tile_pool(name="pq", bufs=3, space="PSUM") as pqpool,
                tc.tile_pool(name="pv", bufs=3, space="PSUM") as pvpool,
            ):
                def emit_q(j):
                    sl = slice(j * 512, (j + 1) * 512)
                    pq = pqpool.tile([P, 512], F32, tag="pq")
                    nc.tensor.matmul(pq[:], wq_s[:], hT[:, sl])
                    # qTs = A_Q*(h@wq) + A_Q*bq  (score psum = schraudolph t)
                    nc.scalar.activation(
                        out=qTs[:, sl],
                        in_=pq[:],
                        func=mybir.ActivationFunctionType.Identity,
                        scale=A_Q,
                        bias=bqs_s,
                    )

                def emit_k(j):
                    sl = slice(j * 512, (j + 1) * 512)
                    pk = pqpool.tile([P, 512], F32, tag="pq")
                    nc.tensor.matmul(pk[:], wk_s[:], hT[:, sl])
                    ki = nc.scalar.activation(
                        out=kT[:, sl],
                        in_=pk[:],
                        func=mybir.ActivationFunctionType.Identity,
                        bias=bk_s,
                    )
                    # HAM pacing through the ACT/DVE-bound stretches of
                    # the projection phase (a >3.4us PE-idle window
                    # re-throttles the array to half clock)
                    pt = pvpool.tile([P, P], F32, tag="pv")
                    ti = nc.tensor.transpose(
                        pt[0:6, :], stats[:, j, :], ident_s
                    )
                    add_dep_helper(
                        ti.ins, ki.ins, sync=False, reason="ham pace"
                    )

                def emit_v(i):
                    # v in fp8, stored pair-interleaved for SwInterleave PV
                    # matmuls. wv arrives with its output channels reversed
                    # (host-side), so the interleaved layout
                    # [A_c127 B_c127 A_c126 ...] is a simple stride-2 byte
                    # write of each chunk's psum. Copies alternate ACT/DVE so
                    # neither queue backlogs into the attention ramp.
                    pv = pvpool.tile([P, C], F32, tag="pv")
                    nc.tensor.matmul(pv[:], hT[:, i * P : (i + 1) * P], wv_s[:])
                    slab = v_all[:, 2 * (i // 2) : 2 * (i // 2) + 2, :]
                    dst = slab.rearrange("p a b -> p (a b)").rearrange(
                        "p (b two) -> p two b", two=2
                    )[:, i % 2, :]
                    if i % 2 == 0:
                        nc.scalar.copy(dst, pv[:])
                    else:
                        nc.vector.tensor_copy(dst, pv[:])

                # just-in-time order: attention pair j of block 0 needs kT
                # chunks 2j..2j+1 (k-block j//2) and v chunks 2j..2j+1, so
                # interleave k and v; later q blocks trail.
                emit_q(0)
                for kb in range(8):
                    emit_k(kb)
                    for i in range(4 * kb, 4 * kb + 4):
                        emit_v(i)
                    if kb % 2 == 0:
                        emit_q(1 + kb // 2)
                for j in range(5, 8):
                    emit_q(j)

            # ---- phase 4: attention over (q-block, k-chunk-pair) steps ----
            # PSUM budget (8 banks): 5 single-bank score tiles + 2 oT + 1
            # shared Z/pop slot.  5 score slots deepen the critical
            # recurrence (score matmul p waits on the exp that frees slot
            # p-2.5) vs 4 slots' p-2.
            with (
                tc.tile_pool(name="sT", bufs=5, space="PSUM") as sTpool,
                tc.tile_pool(name="oT", bufs=2, space="PSUM") as oTpool,
                tc.tile_pool(name="Zp", bufs=1, space="PSUM") as zpool,
            ):
                NSTEP = NQB * NPAIR  # 128 pair-steps
                pexp_tiles = {}
                psum_oT = {}
                psum_Z = {}
                last_score_mm = {}
                last_z_mm = {}

                def emit_scores(p):
                    # Per-half score psums (single PSUM bank each) and
                    # per-half exp: ACT takes half 0, DVE half 1, so each
                    # engine starts as soon as its own matmul lands.
                    qb, j = divmod(p, NPAIR)
                    q0 = qb * QB
                    pexp = pexppool.tile([P, 2, QB], F8, tag="pexp", name=f"pe{p}")
                    pexp_tiles[p] = pexp
                    for h in range(2):
                        kc = 2 * j + h
                        ps = sTpool.tile([P, QB], F32, tag="sT", name=f"sT{p}_{h}")
                        mi = nc.tensor.matmul(
                            ps[:],
                            kT[:, kc * P : (kc + 1) * P],
                            qTs[:, q0 : q0 + QB],
                        )
                        last_score_mm[p] = mi
                        if h == 0:
                            # ACT: exact exp(s - M) into fp8
                            nc.scalar.activation(
                                out=pexp[:, 0, :],
                                in_=ps[:],
                                func=mybir.ActivationFunctionType.Exp,
                                scale=ACT_SCALE,
                                bias=negm_s[:],
                            )
                        else:
                            # DVE: schraudolph bits = sat_u8(max(t + B, 0))
                            nc.vector.tensor_scalar(
                                out=pexp[:, 1, :].bitcast(U8),
                                in0=ps[:],
                                scalar1=B_SCH,
                                scalar2=0.0,
                                op0=mybir.AluOpType.add,
                                op1=mybir.AluOpType.max,
                            )

                def emit_pv(p):
                    qb, j = divmod(p, NPAIR)
                    if j == 0:
                        psum_oT[qb] = oTpool.tile(
                            [P, QB], F32, tag="oT", name=f"oT{qb}"
                        )
                    nc.tensor.matmul(
                        psum_oT[qb][:],
                        v_all[:, 2 * j : 2 * j + 2, :],
                        pexp_tiles[p][:],
                        start=(j == 0),
                        stop=(j == NPAIR - 1),
                        perf_mode=DRSW,
                    )

                def emit_z_sub(qb, g):
                    # Z partial sums, col-tiled: the ones-stationary only has
                    # 32 columns, targeted at PE col-group (m%4) so rounds of
                    # 4 matmuls (distinct col-groups, own XBUS each) execute
                    # concurrently. Partition group 32c..32c+31 of the psum
                    # accumulates the partial Z of halves with m%4==c; the
                    # epilogue sums the four partials with one small matmul.
                    # Sub-bursts for block qb are spread over the next
                    # block's first pairs.
                    if g == 0:
                        psum_Z[qb] = zpool.tile(
                            [P, QB], F32, tag="Z", name=f"Z{qb}"
                        )
                    pz = psum_Z[qb]
                    for m in range(8 * g, 8 * g + 8):
                        jj, h = divmod(m, 2)
                        c = m % 4
                        nc.tensor.matmul(
                            pz[32 * c : 32 * c + 32, :],
                            ones2[:, 0, 0:32],
                            pexp_tiles[qb * NPAIR + jj][:, h, :],
                            start=(m == c),
                            stop=(m == 28 + c),
                            tile_position=(0, 32 * c),
                            # 4 interleaved accumulation chains share this
                            # bank on disjoint partition groups; the zero
                            # region tracker assumes one chain per bank
                            skip_group_check=True,
                        )
                        if h == 1:
                            del pexp_tiles[qb * NPAIR + jj]

                def emit_epilogue(qb, halves=1):
                    poT, pZp = psum_oT.pop(qb), psum_Z.pop(qb)
                    # sum the 4 col-group partial Zs: bf16 copy out, then one
                    # matmul against the group-representative selector
                    Zsb = epipool.tile([P, QB], BF16, tag="Zs", name=f"Zs{qb}")
                    nc.vector.tensor_copy(Zsb[:], pZp[:])
                    pZ = zpool.tile([P, QB], F32, tag="Z", name=f"Zc{qb}")
                    nc.tensor.matmul(pZ[:], sel_s[:], Zsb[:])
                    rZ = epipool.tile([P, QB], F32, tag="rZ", name=f"rZ{qb}")
                    oTn = epipool.tile([P, QB], BF16, tag="oTn", name=f"oTn{qb}")
                    pop = zpool.tile([P, QB], F32, tag="Z", name=f"pop{qb}")
                    outsb = epipool.tile([P, QB], F32, tag="ob", name=f"ob{qb}")
                    HW_ = QB // halves
                    for h in range(halves):
                        hs = slice(h * HW_, (h + 1) * HW_)
                        qsl = slice(qb * QB + h * HW_, qb * QB + (h + 1) * HW_)
                        nc.scalar.activation(
                            out=rZ[:, hs],
                            in_=pZ[:, hs].bitcast(mybir.dt.int32),
                            func=mybir.ActivationFunctionType.Exp,
                            scale=RZ_SCALE,
                            bias=rzb_s[:],
                        )
                        nc.vector.tensor_mul(oTn[:, hs], poT[:, hs], rZ[:, hs])
                        nc.tensor.matmul(pop[:, hs], wo_s[:], oTn[:, hs])
                        # out = (pop + bo2) + xT   (residual + folded biases)
                        nc.vector.scalar_tensor_tensor(
                            out=outsb[:, hs],
                            in0=pop[:, hs],
                            scalar=bo2_s,
                            in1=xT[:, qsl],
                            op0=mybir.AluOpType.add,
                            op1=mybir.AluOpType.add,
                        )
                        nc.sync.dma_start(out_d[:, qsl], outsb[:, hs])

                LA = 3  # pair-steps of score/exp lookahead ahead of PV
                # the Z psum slot is unused until block 1; park ramp-warmup
                # transposes there so the exp-latency stalls of the first
                # pairs don't let the HAM re-throttle the PE to half clock
                wt = zpool.tile([P, P], F32, tag="Z", name="warm")
                for p in range(LA):
                    emit_scores(p)
                for p in range(NSTEP):
                    qb, j = divmod(p, NPAIR)
                    if p < 9:
                        nc.tensor.transpose(
                            wt[0:6, :], stats[:, p % 8, :], ident_s
                        )
                    emit_pv(p)
                    if qb >= 1 and j < 4:
                        emit_z_sub(qb - 1, j)
                    if p + LA < NSTEP:
                        emit_scores(p + LA)
                    if qb >= 1 and j == 10:
                        # delayed so the rZ/mult ops sit late enough in the
                        # in-order ACT/DVE queues not to dam up the exps
                        emit_epilogue(qb - 1)
                # last block: Z sub-bursts and a half-pipelined epilogue on
                # the tail
                for g in range(4):
                    emit_z_sub(NQB - 1, g)
                emit_epilogue(NQB - 1, halves=2)

    nc.compile()
    return nc


_NC_CACHE = {}


def _get_nc():
    if "nc" not in _NC_CACHE:
        _NC_CACHE["nc"] = build_nc()
    return _NC_CACHE["nc"]


def make_in_maps(**inputs):
    bf16 = mybir.dt.np(BF16)
    x = np.ascontiguousarray(np.asarray(inputs["x"], dtype=np.float32))
    ident = np.eye(P, dtype=np.float32)
    gmask = (
        np.kron(np.eye(GROUPS, dtype=np.float32), np.ones((GSIZE, GSIZE), np.float32))
        / GSIZE
    )
    wo64 = np.asarray(inputs["wo"], np.float64)
    bo2 = (
        np.asarray(inputs["bo"], np.float64)
        + np.asarray(inputs["bv"], np.float64) @ wo64
    ).astype(np.float32)
    bqs = (np.asarray(inputs["bq"], np.float64) * A_Q).astype(np.float32)
    # selector summing the 4 col-group partial Zs (rows 0/32/64/96 are the
    # group representatives; every output column gets the same sum)
    sel = np.zeros((P, P), dtype=np.float32)
    sel[[0, 32, 64, 96], :] = 1.0
    sel = sel.astype(bf16)
    consts = np.concatenate(
        [
            ident,
            gmask,
            np.asarray(inputs["gn_scale"], np.float32)[:, None],
            np.asarray(inputs["gn_bias"], np.float32)[:, None],
            bqs[:, None],
            np.asarray(inputs["bk"], np.float32)[:, None],
            bo2[:, None],
        ],
        axis=1,
    )
    shared = {
        "wq": np.asarray(inputs["wq"], np.float32).astype(bf16),
        "wk": np.asarray(inputs["wk"], np.float32).astype(bf16),
        # output channels reversed: the SwInterleave weight layout wants
        # columns in descending order, so the psum comes out pre-reversed
        "wv": np.ascontiguousarray(np.asarray(inputs["wv"], np.float32)[:, ::-1]).astype(bf16),
        "wo": np.asarray(inputs["wo"], np.float32).astype(bf16),
        "consts": np.ascontiguousarray(consts),
        "sel": sel,
    }
    maps = []
    for b in range(B):
        xt = np.ascontiguousarray(x[b].reshape(N, C).T)
        maps.append({"xt": xt, "xtb": xt.astype(bf16), **shared})
    return maps


def kernel(**inputs):
    nc = _get_nc()
    in_maps = make_in_maps(**inputs)
    res = run_bass_kernel_spmd(nc, in_maps, core_ids=list(range(NCORES)))
    out = np.stack(
        [np.asarray(res.results[b]["outT"]).T for b in range(B)], axis=0
    )
    return out.reshape(B, H, W, C).astype(np.float32)


if __name__ == "__main__":
    rng = np.random.default_rng(0)
    ins = {
        "x": rng.standard_normal((B, H, W, C), dtype=np.float32),
        "gn_scale": np.ones(C, np.float32),
        "gn_bias": np.zeros(C, np.float32),
    }
    for w in ("wq", "wk", "wv", "wo"):
        ins[w] = rng.standard_normal((C, C), dtype=np.float32) * SCALE
    for b in ("bq", "bk", "bv", "bo"):
        ins[b] = np.zeros(C, np.float32)
    o = kernel(**ins)
    print("out", o.shape, o.dtype, float(np.abs(o).max()))


# revision 83
# speedup vs baseline: 1.0290x; 1.0290x over previous
"""AttnBlock (GroupNorm + single-head self-attention + residual) on 8 TRN2 cores.

Sharding: data-parallel over batch B=8 -> one [64,64,128] image per core.

Per-core kernel design (fp8/bf16, measured 162-212us depending on the chip's
thermal/power state; was 256us f32r baseline):
  - xT/hT/qT/kT are [C=128 partitions, N=4096 free] (channels on partitions).
  - GN stats run on a bf16 copy of xT (half the DMA to first compute); the
    f32 xT used by the residual is DMA'd behind a semaphore so it never
    contends with the stats-gating stream (~110GB/s per queue).
  - Projections and score matmuls run in bf16: 1 cyc/row like f32r, but the
    LDWEIGHTS is a separate instruction that overlaps the previous matmul,
    unlike f32r whose in-matmul weight load serializes (~230ns/MM).
    (fp8 DoubleRow scores via a c-split [64,2,N] layout measured SLOWER:
    sub-128-partition DR matmuls get no 0.5 cyc/row benefit on hw.)
  - Scores land transposed sT[k, q] = kT_chunk.T @ qT so the probability
    matrix is in [k-partition, q-free] layout for the PV contraction.
  - qT is pre-scaled by A_Q = 8*log2e/sqrt(C) so the score PSUM is directly
    the Schraudolph exponent. Softmax exp splits per half-pair across both
    engines: ACT computes exact exp (scale=ln2/8, bias=-M_SHIFT) into fp8;
    DVE computes Schraudolph fp8 bits = sat_u8(max(psum + B_SCH, 0)) via one
    tensor_scalar with a uint8-bitcast output. M_SHIFT=4 centers exp(s-4) in
    e4m3 range (max observed score ~8.3, overflow at 10.05).
  - Score psums are 5 single-bank tiles; oT 2 banks; Z/pop share 1 bank.
  - PV uses fp8 DoubleRowSwInterleave matmuls (one per k-chunk pair); v is
    written pre-interleaved by reversing wv's columns host-side so each
    chunk's psum copy is a stride-2 byte write (copies alternate ACT/DVE).
  - The softmax denominator Z accumulates via all-ones DoubleRow matmuls in
    sub-bursts of 4 spread over the next block's first pairs; each block's
    epilogue is emitted mid-next-block so its rZ/mult never dam up the
    in-order ACT/DVE queues between exps (that stall was worth ~25us).
  - 1/Z = exp(RZ_SCALE*bitcast_i32(Z) + RZ_BIAS) on ACT: a Schraudolph-log
    feeding the exp table (+-3% on the attention path only). ACT Ln would
    thrash activation-table loads (2.7us each); DVE reciprocal is 3us/tile.
  - The out-projection is transposed: stationary wo, moving oT/Z -> output
    in [C, q] layout, so the residual reads xT directly (no x_all DMA) and
    the epilogue is one scalar_tensor_tensor: out = (pop + bo2) + xT, with
    bo2 = bo + bv@wo folded host-side (kills the v-bias adds too). Output
    DMA writes a transposed [C, N] dram tensor; the host transposes back.
  - Dummy 6-row PE transposes paced behind the GN/projection/ramp phases
    keep the HAM activity window busy (a >3.4us PE-idle gap re-throttles
    the array to 1.2GHz for >=3.4us). Just-in-time k/v/q emission ordering
    removed a 13-16us half-clock window during the attention ramp.
"""

import sys

for _p in ("/opt/trn_rl_repo",):
    if _p not in sys.path:
        sys.path.insert(0, _p)

import numpy as np

import concourse.bass as bass
import concourse.tile as tile
from concourse import bacc, bass_utils, mybir
from concourse.bass_utils import run_bass_kernel_spmd
from concourse.tile import add_dep_helper



B, H, W, C = 8, 64, 64, 128
N = H * W  # 4096 positions per image
GROUPS = 32
GSIZE = C // GROUPS  # 4
EPS = 1e-6
NCORES = 8
P = 128
NT = N // P  # 32 k-chunks
QB = 512  # q-block width
NQB = N // QB  # 8
NPAIR = NT // 2  # 16 k-chunk pairs per q-block
SCALE = C ** -0.5
LOG2E = 1.4426950408889634
M_SHIFT = 4.0  # softmax shift: pexp = exp(s - M_SHIFT)
A_Q = 8.0 * LOG2E * SCALE  # baked into qT so score psum = schraudolph exponent
B_SCH = 8.0 * (7.0 - LOG2E * M_SHIFT) + 0.5  # +0.5 compensates trunc-on-convert
ACT_SCALE = 1.0 / (8.0 * LOG2E)  # un-bake A_Q: exp(psum*ACT_SCALE - M_SHIFT)
LN2 = 0.6931471805599453
# 1/Z ~= exp(-ln2*(bits(Z)*2^-23 - 127.0450466)): schraudolph-log feeding the
# exp table (stays in the exp function set; ACT Ln would thrash table loads
# and DVE reciprocal measures ~3us per 512-elem tile). Max rel err ~3%,
# affecting only the attention path (~13% of output norm).
RZ_SCALE = -LN2 / (1 << 23)
RZ_BIAS = LN2 * (127.0 - 0.0450466)

F32 = mybir.dt.float32
BF16 = mybir.dt.bfloat16
F8 = mybir.dt.float8e4
U8 = mybir.dt.uint8
DR = mybir.MatmulPerfMode.DoubleRow
DRSW = mybir.MatmulPerfMode.DoubleRowSwInterleave


def build_nc():
    nc = bacc.Bacc("TRN2", target_bir_lowering=False, debug=False)

    xt_d = nc.dram_tensor("xt", [C, N], F32, kind="ExternalInput")
    xtb_d = nc.dram_tensor("xtb", [C, N], BF16, kind="ExternalInput")
    wq_d = nc.dram_tensor("wq", [C, C], BF16, kind="ExternalInput")
    wk_d = nc.dram_tensor("wk", [C, C], BF16, kind="ExternalInput")
    wv_d = nc.dram_tensor("wv", [C, C], BF16, kind="ExternalInput")
    wo_d = nc.dram_tensor("wo", [C, C], BF16, kind="ExternalInput")
    # one packed constants tensor: [ident | gmask | gns gnb bqs bk bo2]
    # (seven separate small DMAs cost ~600ns of queue-issue time each)
    consts_d = nc.dram_tensor("consts", [P, 2 * P + 5], F32, kind="ExternalInput")
    out_d = nc.dram_tensor("outT", [C, N], F32, kind="ExternalOutput")

    def col(ap_1d):
        # [C] dram -> [C, 1] partition-column view
        return ap_1d.unsqueeze(1)

    with tile.TileContext(nc) as tc:
        with (
            tc.tile_pool(name="persist", bufs=1) as data,
            tc.tile_pool(name="small", bufs=1) as small,
            tc.tile_pool(name="pexp", bufs=NPAIR + 7) as pexppool,
            tc.tile_pool(name="epi", bufs=3) as epipool,
        ):
            # ---- persistent SBUF tiles ----
            xT = data.tile([P, N], F32)  # exact residual (read late)
            xTb = data.tile([P, N], BF16)  # stats + groupnorm input
            hT = data.tile([P, N], BF16)
            # q/k stay bf16: fp8 DoubleRow scores via a c-split [64,2,N]
            # layout measured SLOWER on hw (64-partition DR matmuls use half
            # the array and get no 0.5-cyc/row benefit: 685ns vs 389ns)
            qTs = data.tile([P, N], BF16)  # q, pre-scaled by A_Q
            kT = data.tile([P, N], BF16)
            v_all = data.tile([P, NT, C], F8)

            wq_s = small.tile([C, C], BF16)
            wk_s = small.tile([C, C], BF16)
            wv_s = small.tile([C, C], BF16)
            wo_s = small.tile([C, C], BF16)
            consts_s = small.tile([P, 2 * P + 5], F32)
            ident_s = consts_s[:, 0:P]
            gmask_s = consts_s[:, P : 2 * P]
            gns_s = consts_s[:, 2 * P : 2 * P + 1]
            gnb_s = consts_s[:, 2 * P + 1 : 2 * P + 2]
            bqs_s = consts_s[:, 2 * P + 2 : 2 * P + 3]
            bk_s = consts_s[:, 2 * P + 3 : 2 * P + 4]
            bo2_s = consts_s[:, 2 * P + 4 : 2 * P + 5]
            ones2 = small.tile([P, 2, C], F8)
            eps_s = small.tile([C, 1], F32)
            negm_s = small.tile([C, 1], F32)
            rzb_s = small.tile([C, 1], F32)

            # xTb (bf16, half the bytes) gates the GN stats chain: the first
            # two 256-col chunks are small so bn_stats starts ASAP, the rest
            # stream wide. The exact f32 xT is only read by the residual
            # epilogues tens of microseconds later, so it streams afterwards.
            nc.gpsimd.dma_start(consts_s[:], consts_d[:])
            xtb_cuts = [0, 512, 1408, 2304, 3200, N]
            for ci in range(5):
                cs = slice(xtb_cuts[ci], xtb_cuts[ci + 1])
                eng = nc.sync if ci % 2 == 0 else nc.gpsimd
                eng.dma_start(xTb[:, cs], xtb_d[:, cs])
            nc.gpsimd.dma_start(wq_s[:], wq_d[:])
            nc.gpsimd.dma_start(wk_s[:], wk_d[:])
            nc.gpsimd.dma_start(wv_s[:], wv_d[:])
            nc.gpsimd.dma_start(wo_s[:], wo_d[:])
            nc.gpsimd.memset(ones2[:], 1.0)
            nc.vector.memset(eps_s[:], EPS)
            nc.vector.memset(negm_s[:], -M_SHIFT)
            nc.vector.memset(rzb_s[:], RZ_BIAS)

            # ---- phase 1+2: group norm stats straight off the xT DMA ----
            stats = small.tile([P, 8, nc.vector.BN_STATS_DIM], F32)
            with tc.tile_pool(name="tp", bufs=3, space="PSUM") as tpsum:
                stat_is = []
                for j in range(8):
                    si = nc.vector.bn_stats(
                        out=stats[:, j, :], in_=xTb[:, j * 512 : (j + 1) * 512]
                    )
                    stat_is.append(si)
                    if j % 3 != 0:
                        continue
                    # keep the PE's HAM activity monitor busy through the
                    # DVE-bound stats/GN window so the attention matmuls
                    # start at full clock (idle >3.4us re-throttles); one
                    # dummy transpose every ~2us of stats suffices.
                    pt = tpsum.tile([P, P], F32, tag="tp")
                    nc.tensor.transpose(
                        pt[0:6, :], stats[:, j, :], ident_s
                    )
                # f32 xT streams only after the stats-gating xtb is nearly
                # done: both share ~110GB/s per DMA queue and the epilogues
                # that read xT start tens of microseconds later.
                for ci in range(4):
                    cs = slice(ci * N // 4, (ci + 1) * N // 4)
                    eng = nc.sync if ci % 2 == 0 else nc.gpsimd
                    di = eng.dma_start(xT[:, cs], xt_d[:, cs])
                    add_dep_helper(
                        di.ins, stat_is[5].ins, sync=True, reason="xt after xtb"
                    )
                mv = small.tile([P, nc.vector.BN_AGGR_DIM], F32)
                nc.vector.bn_aggr(out=mv[:], in_=stats[:])
                # per-channel [mean, E[x^2]] -> group-averaged via mask matmul
                st2 = small.tile([P, 2], F32)
                nc.vector.tensor_copy(st2[:, 0:1], mv[:, 0:1])
                msq = small.tile([P, 1], F32)
                nc.vector.tensor_mul(msq[:], mv[:, 0:1], mv[:, 0:1])
                nc.vector.tensor_add(st2[:, 1:2], mv[:, 1:2], msq[:])
                gpsum = tpsum.tile([P, 2], F32, tag="tp")
                nc.tensor.matmul(gpsum[:], gmask_s, st2[:])
                gstat = small.tile([P, 2], F32)
                nc.vector.tensor_copy(gstat[:], gpsum[:])

                # var_g = E_g[x^2] - mean_g^2 ; rstd = 1/sqrt(var_g + eps)
                varg = small.tile([P, 1], F32)
                nc.vector.tensor_mul(varg[:], gstat[:, 0:1], gstat[:, 0:1])
                nc.vector.tensor_tensor(
                    varg[:], gstat[:, 1:2], varg[:], mybir.AluOpType.subtract
                )
                nc.scalar.activation(
                    out=varg[:],
                    in_=varg[:],
                    func=mybir.ActivationFunctionType.Sqrt,
                    bias=eps_s[:],
                    scale=1.0,
                )
                rstd = small.tile([P, 1], F32)
                nc.vector.reciprocal(rstd[:], varg[:])
                # h = x * A + Bc with A = rstd*scale, Bc = bias - mean*A
                A_s = small.tile([P, 1], F32)
                nc.vector.tensor_mul(A_s[:], rstd[:], gns_s)
                mA = small.tile([P, 1], F32)
                nc.vector.tensor_mul(mA[:], gstat[:, 0:1], A_s[:])
                Bc_s = small.tile([P, 1], F32)
                nc.vector.tensor_tensor(
                    Bc_s[:], gnb_s, mA[:], mybir.AluOpType.subtract
                )
                # hT (bf16) in 8 chunks; alternate ACT and DVE.  A dummy PE
                # transpose paced behind each chunk keeps the HAM activity
                # window busy through this PE-idle stretch (else the array
                # re-throttles to half clock right as projections start).
                for j in range(8):
                    sl = slice(j * 512, (j + 1) * 512)
                    if j % 2 == 0:
                        hi = nc.scalar.activation(
                            out=hT[:, sl],
                            in_=xTb[:, sl],
                            func=mybir.ActivationFunctionType.Identity,
                            scale=A_s[:],
                            bias=Bc_s[:],
                        )
                    else:
                        # gpsimd is SBUF-only but this op is SBUF->SBUF
                        eng = nc.gpsimd if j % 4 == 1 else nc.vector
                        hi = eng.tensor_scalar(
                            out=hT[:, sl],
                            in0=xTb[:, sl],
                            scalar1=A_s[:],
                            scalar2=Bc_s[:],
                            op0=mybir.AluOpType.mult,
                            op1=mybir.AluOpType.add,
                        )
                    pt = tpsum.tile([P, P], F32, tag="tp")
                    ti = nc.tensor.transpose(
                        pt[0:6, :], stats[:, j, :], ident_s
                    )
                    add_dep_helper(
                        ti.ins, hi.ins, sync=False, reason="ham pace"
                    )

            # ---- phase 3: projections qTs/kT [C,N] bf16, v [pos,C] fp8 ----
            with (
                tc.tile_pool(name="pq", bufs=3, space="PSUM") as pqpool,
                tc.tile_pool(name="pv", bufs=3, space="PSUM") as pvpool,
            ):
                def emit_q(j):
                    sl = slice(j * 512, (j + 1) * 512)
                    pq = pqpool.tile([P, 512], F32, tag="pq")
                    nc.tensor.matmul(pq[:], wq_s[:], hT[:, sl])
                    # qTs = A_Q*(h@wq) + A_Q*bq  (score psum = schraudolph t)
                    nc.scalar.activation(
                        out=qTs[:, sl],
                        in_=pq[:],
                        func=mybir.ActivationFunctionType.Identity,
                        scale=A_Q,
                        bias=bqs_s,
                    )

                def emit_k(j):
                    sl = slice(j * 512, (j + 1) * 512)
                    pk = pqpool.tile([P, 512], F32, tag="pq")
                    nc.tensor.matmul(pk[:], wk_s[:], hT[:, sl])
                    ki = nc.scalar.activation(
                        out=kT[:, sl],
                        in_=pk[:],
                        func=mybir.ActivationFunctionType.Identity,
                        bias=bk_s,
                    )
                    # HAM pacing through the ACT/DVE-bound stretches of
                    # the projection phase (a >3.4us PE-idle window
                    # re-throttles the array to half clock)
                    pt = pvpool.tile([P, P], F32, tag="pv")
                    ti = nc.tensor.transpose(
                        pt[0:6, :], stats[:, j, :], ident_s
                    )
                    add_dep_helper(
                        ti.ins, ki.ins, sync=False, reason="ham pace"
                    )

                def emit_v(i):
                    # v in fp8, stored pair-interleaved for SwInterleave PV
                    # matmuls. wv arrives with its output channels reversed
                    # (host-side), so the interleaved layout
                    # [A_c127 B_c127 A_c126 ...] is a simple stride-2 byte
                    # write of each chunk's psum. Copies alternate ACT/DVE so
                    # neither queue backlogs into the attention ramp.
                    pv = pvpool.tile([P, C], F32, tag="pv")
                    nc.tensor.matmul(pv[:], hT[:, i * P : (i + 1) * P], wv_s[:])
                    slab = v_all[:, 2 * (i // 2) : 2 * (i // 2) + 2, :]
                    dst = slab.rearrange("p a b -> p (a b)").rearrange(
                        "p (b two) -> p two b", two=2
                    )[:, i % 2, :]
                    if i % 2 == 0:
                        nc.scalar.copy(dst, pv[:])
                    else:
                        nc.vector.tensor_copy(dst, pv[:])

                # just-in-time order: attention pair j of block 0 needs kT
                # chunks 2j..2j+1 (k-block j//2) and v chunks 2j..2j+1, so
                # interleave k and v; later q blocks trail.
                emit_q(0)
                for kb in range(8):
                    emit_k(kb)
                    for i in range(4 * kb, 4 * kb + 4):
                        emit_v(i)
                    if kb % 2 == 0:
                        emit_q(1 + kb // 2)
                for j in range(5, 8):
                    emit_q(j)

            # ---- phase 4: attention over (q-block, k-chunk-pair) steps ----
            # PSUM budget (8 banks): 5 single-bank score tiles + 2 oT + 1
            # shared Z/pop slot.  5 score slots deepen the critical
            # recurrence (score matmul p waits on the exp that frees slot
            # p-2.5) vs 4 slots' p-2.
            with (
                tc.tile_pool(name="sT", bufs=5, space="PSUM") as sTpool,
                tc.tile_pool(name="oT", bufs=2, space="PSUM") as oTpool,
                tc.tile_pool(name="Zp", bufs=1, space="PSUM") as zpool,
            ):
                NSTEP = NQB * NPAIR  # 128 pair-steps
                pexp_tiles = {}
                psum_oT = {}
                psum_Z = {}
                last_score_mm = {}
                last_z_mm = {}

                def emit_scores(p):
                    # Per-half score psums (single PSUM bank each) and
                    # per-half exp: ACT takes half 0, DVE half 1, so each
                    # engine starts as soon as its own matmul lands.
                    qb, j = divmod(p, NPAIR)
                    q0 = qb * QB
                    pexp = pexppool.tile([P, 2, QB], F8, tag="pexp", name=f"pe{p}")
                    pexp_tiles[p] = pexp
                    for h in range(2):
                        kc = 2 * j + h
                        ps = sTpool.tile([P, QB], F32, tag="sT", name=f"sT{p}_{h}")
                        mi = nc.tensor.matmul(
                            ps[:],
                            kT[:, kc * P : (kc + 1) * P],
                            qTs[:, q0 : q0 + QB],
                        )
                        last_score_mm[p] = mi
                        if h == 0:
                            # ACT: exact exp(s - M) into fp8
                            nc.scalar.activation(
                                out=pexp[:, 0, :],
                                in_=ps[:],
                                func=mybir.ActivationFunctionType.Exp,
                                scale=ACT_SCALE,
                                bias=negm_s[:],
                            )
                        else:
                            # DVE: schraudolph bits = sat_u8(max(t + B, 0))
                            nc.vector.tensor_scalar(
                                out=pexp[:, 1, :].bitcast(U8),
                                in0=ps[:],
                                scalar1=B_SCH,
                                scalar2=0.0,
                                op0=mybir.AluOpType.add,
                                op1=mybir.AluOpType.max,
                            )

                def emit_pv(p):
                    qb, j = divmod(p, NPAIR)
                    if j == 0:
                        psum_oT[qb] = oTpool.tile(
                            [P, QB], F32, tag="oT", name=f"oT{qb}"
                        )
                    nc.tensor.matmul(
                        psum_oT[qb][:],
                        v_all[:, 2 * j : 2 * j + 2, :],
                        pexp_tiles[p][:],
                        start=(j == 0),
                        stop=(j == NPAIR - 1),
                        perf_mode=DRSW,
                    )

                def emit_z_sub(qb, g):
                    # Z sub-burst g: 4 DoubleRow matmuls against the all-ones
                    # stationary (one LDWEIGHTS per burst). Sub-bursts for
                    # block qb are spread over the next block's first pairs so
                    # neither the PE nor the ACT/DVE queues see one long
                    # block-boundary stall.
                    if g == 0:
                        psum_Z[qb] = zpool.tile(
                            [P, QB], F32, tag="Z", name=f"Z{qb}"
                        )
                    for j in range(4 * g, 4 * g + 4):
                        nc.tensor.matmul(
                            psum_Z[qb][:],
                            ones2[:],
                            pexp_tiles[qb * NPAIR + j][:],
                            start=(j == 0),
                            stop=(j == NPAIR - 1),
                            perf_mode=DR,
                        )
                        del pexp_tiles[qb * NPAIR + j]

                def emit_epilogue(qb, halves=1):
                    poT, pZ = psum_oT.pop(qb), psum_Z.pop(qb)
                    rZ = epipool.tile([P, QB], F32, tag="rZ", name=f"rZ{qb}")
                    oTn = epipool.tile([P, QB], BF16, tag="oTn", name=f"oTn{qb}")
                    pop = zpool.tile([P, QB], F32, tag="Z", name=f"pop{qb}")
                    outsb = epipool.tile([P, QB], F32, tag="ob", name=f"ob{qb}")
                    HW_ = QB // halves
                    for h in range(halves):
                        hs = slice(h * HW_, (h + 1) * HW_)
                        qsl = slice(qb * QB + h * HW_, qb * QB + (h + 1) * HW_)
                        nc.scalar.activation(
                            out=rZ[:, hs],
                            in_=pZ[:, hs].bitcast(mybir.dt.int32),
                            func=mybir.ActivationFunctionType.Exp,
                            scale=RZ_SCALE,
                            bias=rzb_s[:],
                        )
                        nc.vector.tensor_mul(oTn[:, hs], poT[:, hs], rZ[:, hs])
                        nc.tensor.matmul(pop[:, hs], wo_s[:], oTn[:, hs])
                        # out = (pop + bo2) + xT   (residual + folded biases)
                        nc.vector.scalar_tensor_tensor(
                            out=outsb[:, hs],
                            in0=pop[:, hs],
                            scalar=bo2_s,
                            in1=xT[:, qsl],
                            op0=mybir.AluOpType.add,
                            op1=mybir.AluOpType.add,
                        )
                        nc.sync.dma_start(out_d[:, qsl], outsb[:, hs])

                LA = 3  # pair-steps of score/exp lookahead ahead of PV
                # the Z psum slot is unused until block 1; park ramp-warmup
                # transposes there so the exp-latency stalls of the first
                # pairs don't let the HAM re-throttle the PE to half clock
                wt = zpool.tile([P, P], F32, tag="Z", name="warm")
                for p in range(LA):
                    emit_scores(p)
                for p in range(NSTEP):
                    qb, j = divmod(p, NPAIR)
                    if p < 9:
                        nc.tensor.transpose(
                            wt[0:6, :], stats[:, p % 8, :], ident_s
                        )
                    emit_pv(p)
                    if qb >= 1 and j < 4:
                        emit_z_sub(qb - 1, j)
                    if p + LA < NSTEP:
                        emit_scores(p + LA)
                    if qb >= 1 and j == 10:
                        # delayed so the rZ/mult ops sit late enough in the
                        # in-order ACT/DVE queues not to dam up the exps
                        emit_epilogue(qb - 1)
                # last block: Z sub-bursts and a half-pipelined epilogue on
                # the tail
                for g in range(4):
                    emit_z_sub(NQB - 1, g)
                emit_epilogue(NQB - 1, halves=2)

    nc.compile()
    return nc


_NC_CACHE = {}


def _get_nc():
    if "nc" not in _NC_CACHE:
        _NC_CACHE["nc"] = build_nc()
    return _NC_CACHE["nc"]


def make_in_maps(**inputs):
    bf16 = mybir.dt.np(BF16)
    x = np.ascontiguousarray(np.asarray(inputs["x"], dtype=np.float32))
    ident = np.eye(P, dtype=np.float32)
    gmask = (
        np.kron(np.eye(GROUPS, dtype=np.float32), np.ones((GSIZE, GSIZE), np.float32))
        / GSIZE
    )
    wo64 = np.asarray(inputs["wo"], np.float64)
    bo2 = (
        np.asarray(inputs["bo"], np.float64)
        + np.asarray(inputs["bv"], np.float64) @ wo64
    ).astype(np.float32)
    bqs = (np.asarray(inputs["bq"], np.float64) * A_Q).astype(np.float32)
    consts = np.concatenate(
        [
            ident,
            gmask,
            np.asarray(inputs["gn_scale"], np.float32)[:, None],
            np.asarray(inputs["gn_bias"], np.float32)[:, None],
            bqs[:, None],
            np.asarray(inputs["bk"], np.float32)[:, None],
            bo2[:, None],
        ],
        axis=1,
    )
    shared = {
        "wq": np.asarray(inputs["wq"], np.float32).astype(bf16),
        "wk": np.asarray(inputs["wk"], np.float32).astype(bf16),
        # output channels reversed: the SwInterleave weight layout wants
        # columns in descending order, so the psum comes out pre-reversed
        "wv": np.ascontiguousarray(np.asarray(inputs["wv"], np.float32)[:, ::-1]).astype(bf16),
        "wo": np.asarray(inputs["wo"], np.float32).astype(bf16),
        "consts": np.ascontiguousarray(consts),
    }
    maps = []
    for b in range(B):
        xt = np.ascontiguousarray(x[b].reshape(N, C).T)
        maps.append({"xt": xt, "xtb": xt.astype(bf16), **shared})
    return maps


def kernel(**inputs):
    nc = _get_nc()
    in_maps = make_in_maps(**inputs)
    res = run_bass_kernel_spmd(nc, in_maps, core_ids=list(range(NCORES)))
    out = np.stack(
        [np.asarray(res.results[b]["outT"]).T for b in range(B)], axis=0
    )
    return out.reshape(B, H, W, C).astype(np.float32)


if __name__ == "__main__":
    rng = np.random.default_rng(0)
    ins = {
        "x": rng.standard_normal((B, H, W, C), dtype=np.float32),
        "gn_scale": np.ones(C, np.float32),
        "gn_bias": np.zeros(C, np.float32),
    }
    for w in ("wq", "wk", "wv", "wo"):
        ins[w] = rng.standard_normal((C, C), dtype=np.float32) * SCALE
    for b in ("bq", "bk", "bv", "bo"):
        ins[b] = np.zeros(C, np.float32)
    o = kernel(**ins)
    print("out", o.shape, o.dtype, float(np.abs(o).max()))
